# revision 31
# baseline (speedup 1.0000x reference)
"""Banded (sliding-window) multi-head attention for Trainium2, 8 NeuronCores.

Problem: x[4, 2048, 512] -> QKV proj -> RoPE -> banded attention
(window [q-127, q+128]) -> out proj.  See reference.py.

Sharding: (batch n, head-half) -> 8 cores.  Each core computes 4 heads of one
batch end-to-end and a partial out-projection (contraction over its 256 hidden
dims); host gather sums the two partials per batch and adds the bias.

v2 pipeline (all matmuls bf16 with fp32 PSUM accumulation):
  - 5 batched input DMAs (constants host-packed into two [128, 4224] tensors).
  - qkT = Wqk^T.T @ xT in psum; RoPE via rot(u*sin) == rot(u)*sin:
      t1 = ps*cos (DVE), t2 = ps*sin (Pool), ps <- R @ t2 (PE, in-place),
      qkT = t1 + ps (DVE).  No separate raw copy.
  - v token-major via xT.T @ WvT, ones column -> denominators.
  - scores chunk-grouped: per (key chunk j, head pack, head a) one matmul
    kT_j^T.T @ qT[3-tile window] with N<=384, psum [128, 2a, 512pad].
  - exp per (chunk, hp) over both heads in one strided ACT call.
  - band masks: per-block batched affine_selects on the exp ring (Pool).
  - attn[q, 65] accumulated over 3 chunks; transpose written in-place into
    the dead tail of the same psum bank; out proj -> bf16 out, block DMAs.
"""

import numpy as np
import ml_dtypes

import concourse.bass as bass
import concourse.bacc as bacc
import concourse.mybir as mybir
import concourse.tile as tile
from concourse import bass_utils

# ---------------- problem constants (hardcoded per contract) ----------------
N_BATCH = 4
T = 2048
D_MODEL = 512
NHEAD = 8
HEAD_DIM = 64           # also rotary dim
WIN_LO, WIN_HI = 127, 128
N_CORES = 8

NT = T // 128           # 16 query tiles / key chunks of 128
BF = mybir.dt.bfloat16
F32 = mybir.dt.float32

# c1 column layout: wqk [4x512] | wv [4x256] | cos0 [512] | sin0 [512] | rt [128]
C1_WQK, C1_WV, C1_COS, C1_SIN, C1_RT = 0, 2048, 3072, 3584, 4096
C1_COLS = 4224
# c2 layout: ow [2x512] | cos rest [1536] | sin rest [1536] | maskL | maskR
C2_OW, C2_COS, C2_SIN, C2_ML, C2_MR = 0, 1024, 2560, 4096, 4224
C2_COLS = 4352

_CACHE = {}


# ---------------- host-side constant prep ----------------
def _bf16(a):
    return np.ascontiguousarray(a, dtype=np.float32).astype(ml_dtypes.bfloat16)


def _rope_tables():
    # row p of a 128-partition head-pack corresponds to head dim d = p % 64
    d_idx = np.arange(128) % HEAD_DIM
    f_idx = d_idx % (HEAD_DIM // 2)
    invf = 1.0 / (10000.0 ** (np.arange(0, HEAD_DIM, 2, dtype=np.float32) / HEAD_DIM))
    ang = np.arange(T, dtype=np.float32)[None, :] * invf[f_idx][:, None]  # [128, T]
    return np.cos(ang, dtype=np.float32), np.sin(ang, dtype=np.float32)


def _rot_matrix_T():
    # ps_rot = R @ t2 with R the rotate_half signed permutation (per 64-dim
    # head); lhsT convention means we pass R.T
    R = np.zeros((128, 128), np.float32)
    for p in range(128):
        if p % 64 < 32:
            R[p, p + 32] = -1.0
        else:
            R[p, p - 32] = 1.0
    return R.T


def _prep_consts(Wqkv_w, out_w, half):
    hs = [half * 4 + i for i in range(4)]
    Wq = Wqkv_w[0 * D_MODEL:1 * D_MODEL].reshape(NHEAD, HEAD_DIM, D_MODEL)[hs]
    Wk = Wqkv_w[1 * D_MODEL:2 * D_MODEL].reshape(NHEAD, HEAD_DIM, D_MODEL)[hs]
    Wv = Wqkv_w[2 * D_MODEL:3 * D_MODEL].reshape(NHEAD, HEAD_DIM, D_MODEL)[hs]
    Wq = Wq * np.float32(1.0 / np.sqrt(HEAD_DIM))     # fold attention scale
    # feat order: q(h0,h1), q(h2,h3), k(h0,h1), k(h2,h3)
    wqk = np.concatenate([Wq.reshape(2, 128, D_MODEL), Wk.reshape(2, 128, D_MODEL)], 0)
    wqkT = wqk.reshape(512, D_MODEL).T.reshape(4, 128, 512)       # [xfeat c, 128, feat]
    wvT = Wv.reshape(256, D_MODEL).T.reshape(4, 128, 256)
    owT = out_w[:, half * 256:(half + 1) * 256].T.reshape(2, 128, 512)

    cos, sin = _rope_tables()
    rtT = _rot_matrix_T()
    c1 = np.concatenate([
        wqkT.transpose(1, 0, 2).reshape(128, 2048),
        wvT.transpose(1, 0, 2).reshape(128, 1024),
        cos[:, 0:512], sin[:, 0:512], rtT,
    ], axis=1)
    kp = np.arange(128)[:, None]
    qp = np.arange(128)[None, :]
    c2 = np.concatenate([
        owT.transpose(1, 0, 2).reshape(128, 1024),
        cos[:, 512:], sin[:, 512:],
        (kp >= qp + 1).astype(np.float32),   # left-role keep mask
        (kp <= qp).astype(np.float32),       # right-role keep mask
    ], axis=1)
    assert c1.shape[1] == C1_COLS and c2.shape[1] == C2_COLS
    return _bf16(c1), _bf16(c2)


# ---------------- bass program ----------------
def build_nc(reps=1):
    """reps>1 repeats the whole kernel body (timing harness only)."""
    import os
    KDEBUG = os.environ.get("KDEBUG") == "1"
    PHASE = os.environ.get("KBUILD_PHASE", "full")  # prod | noattn | full
    nc = bacc.Bacc("TRN2", debug=False, enable_asserts=False,
                   target_bir_lowering=False)

    c1_d = nc.dram_tensor("c1", [128, C1_COLS], BF, kind="ExternalInput")
    c2_d = nc.dram_tensor("c2", [128, C2_COLS], BF, kind="ExternalInput")
    x0_d = nc.dram_tensor("x0", [128, 4, 512], BF, kind="ExternalInput")
    xr_d = nc.dram_tensor("xr", [128, 4, 1536], BF, kind="ExternalInput")
    id_d = nc.dram_tensor("ident32", [128, 128], F32, kind="ExternalInput")
    out_d = nc.dram_tensor("out", [T, 512], BF, kind="ExternalOutput")
    if KDEBUG:
        dbg_qT = nc.dram_tensor("dbg_qT", [128, 2, T], BF, kind="ExternalOutput")
        dbg_kT = nc.dram_tensor("dbg_kT", [128, 2, T], BF, kind="ExternalOutput")
        dbg_v4 = nc.dram_tensor("dbg_v4", [128, 4, NT, 65], BF, kind="ExternalOutput")
        dbg_exp = nc.dram_tensor("dbg_exp", [4, 128, 4, 2, 2, 384], BF, kind="ExternalOutput")

    with tile.TileContext(nc) as tc:
        with (
            tc.tile_pool(name="persist", bufs=1) as pers,
            tc.tile_pool(name="rawp", bufs=3) as rawp,
            tc.tile_pool(name="t1p", bufs=3) as t1p,
            tc.tile_pool(name="t2p", bufs=3) as t2p,
            tc.tile_pool(name="rcpp", bufs=4) as rcpp,
            tc.tile_pool(name="aqp", bufs=6) as aqp,
            tc.tile_pool(name="attp", bufs=3) as attp,
            tc.tile_pool(name="expp", bufs=4) as expp,
            tc.tile_pool(name="pscore", bufs=2, space="PSUM") as pscore,
            tc.tile_pool(name="pprod", bufs=2, space="PSUM") as pprod,
            tc.tile_pool(name="pmix", bufs=2, space="PSUM") as pmix,
        ):
            # ------- persistent SBUF tensors -------
            c1 = pers.tile([128, C1_COLS], BF)
            c2 = pers.tile([128, C2_COLS], BF)
            ident = pers.tile([128, 128], F32)
            xT = pers.tile([128, 4, T], BF)
            qT = pers.tile([128, 2, T], BF)
            kT = pers.tile([128, 2, T], BF)
            v4 = pers.tile([128, 4, NT, 65], BF)
            osb = pers.tile([128, 4, 512], BF)

            def cos_ap(n):
                return (c1[:, C1_COS:C1_COS + 512] if n == 0
                        else c2[:, C2_COS + (n - 1) * 512:C2_COS + n * 512])

            def sin_ap(n):
                return (c1[:, C1_SIN:C1_SIN + 512] if n == 0
                        else c2[:, C2_SIN + (n - 1) * 512:C2_SIN + n * 512])

            rep_ctx = tc.For_i(0, reps, 1) if reps > 1 else None
            if rep_ctx is not None:
                rep_ctx.__enter__()
            for rep in range(1):
                # ------- input DMAs (few, large) -------
                # issue input DMAs from separate queues so descriptor
                # generation and transfers overlap from t=0
                nc.sync.dma_start(out=c1[:, 0:3072], in_=c1_d[:, 0:3072])
                nc.sync.dma_start(out=xT[:, :, 0:512], in_=x0_d[:])
                nc.sync.dma_start(out=xT[:, :, 512:T], in_=xr_d[:])
                nc.sync.dma_start(out=c1[:, 3072:], in_=c1_d[:, 3072:])
                nc.sync.dma_start(out=c2[:], in_=c2_d[:])
                nc.sync.dma_start(out=ident[:], in_=id_d[:])

                nc.vector.memset(v4[:, :, :, 64:65], 1.0)  # ones col -> sums

                # per-step state carried between pipeline stages
                raws = [None] * 16          # raw qk [128,512] sbuf bf16 per m-idx
                t1s = [None] * 16           # raw*cos
                exp_blocks = [None] * 4
                pa_tiles = {}               # (t, hp) -> psum attn tile
                att_tiles = {}              # t -> sbuf attnT tile

                # stage A: qk production matmuls + raw copy + cos mul
                def qk_mms(s):
                    n, m = s // 4, s % 4
                    nsl = slice(n * 512, (n + 1) * 512)
                    ps = pprod.tile([128, 512], F32, tag="prod", name=f"qk{s}")
                    for c in range(4):
                        nc.tensor.matmul(
                            ps[:],
                            lhsT=c1[:, C1_WQK + c * 512 + m * 128:
                                    C1_WQK + c * 512 + (m + 1) * 128],
                            rhs=xT[:, c, nsl],
                            start=(c == 0), stop=(c == 3),
                        )
                    raw = rawp.tile([128, 512], BF, tag="raw")
                    if s < 8:
                        nc.vector.tensor_copy(raw[:], ps[:])
                    else:
                        nc.scalar.copy(raw[:], ps[:])
                    t1 = t1p.tile([128, 512], BF, tag="t1")
                    nc.gpsimd.tensor_mul(t1[:], raw[:], cos_ap(n))
                    raws[s], t1s[s] = raw, t1

                # stage B (one step later): rotate on PE, sin mul on DVE
                t2s = [None] * 16

                def rope_rot(s):
                    n = s // 4
                    psr = pprod.tile([128, 512], F32, tag="prod", name=f"rot{s}")
                    nc.tensor.matmul(psr[:], lhsT=c1[:, C1_RT:C1_RT + 128],
                                     rhs=raws[s][:], start=True, stop=True)
                    t2 = t2p.tile([128, 512], BF, tag="t2")
                    nc.vector.tensor_mul(t2[:], psr[:], sin_ap(n))
                    t2s[s] = t2
                    raws[s] = None

                # stage C (two steps later): combine on Pool
                def rope_add(s):
                    n, m = s // 4, s % 4
                    nsl = slice(n * 512, (n + 1) * 512)
                    dest = qT[:, m, nsl] if m < 2 else kT[:, m - 2, nsl]
                    nc.gpsimd.tensor_add(dest, t1s[s][:], t2s[s][:])
                    t1s[s] = t2s[s] = None

                def v_production(t):
                    ps = pprod.tile([128, 512], F32, tag="prod", name=f"v{t}")
                    for c in range(4):
                        nc.tensor.matmul(
                            ps[:, 0:256],
                            lhsT=xT[:, c, t * 128:(t + 1) * 128],
                            rhs=c1[:, C1_WV + c * 256:C1_WV + (c + 1) * 256],
                            start=(c == 0), stop=(c == 3),
                        )
                    nc.scalar.copy(
                        v4[:, :, t, 0:64],
                        ps[:, 0:256].rearrange("p (h d) -> p h d", h=4),
                    )

                # scores matmuls for key chunk j (exp runs next step)
                score_ps = {}

                def chunk_cols(j):
                    q0 = max(j - 1, 0) * 128
                    q1 = min(j + 2, NT) * 128
                    return q0, q1, q0 - (j - 1) * 128

                def scores_chunk(j):
                    q0, q1, col0 = chunk_cols(j)
                    ncols = q1 - q0
                    for hp in range(2):
                        ps = pscore.tile([128, 1024], F32, tag="score",
                                         name=f"sc{j}_{hp}")
                        score_ps[(j, hp)] = ps
                        for a in range(2):
                            rsl = slice(a * 64, (a + 1) * 64)
                            nc.tensor.matmul(
                                ps[:, a * 512 + col0:a * 512 + col0 + ncols],
                                lhsT=kT[rsl, hp, j * 128:(j + 1) * 128],
                                rhs=qT[rsl, hp, q0:q1],
                                start=True, stop=True,
                            )

                def exp_chunk(j):
                    b, jj = j // 4, j % 4
                    if jj == 0:
                        exp_blocks[b] = expp.tile([128, 4, 2, 2, 384], BF,
                                                  tag="exp", name=f"exp{b}")
                    eb = exp_blocks[b]
                    if KDEBUG and j == 0:
                        nc.vector.memset(eb[:, 0, :, :, 0:128], 0.0)
                    if KDEBUG and j == NT - 1:
                        nc.vector.memset(eb[:, 3, :, :, 256:384], 0.0)
                    q0, q1, col0 = chunk_cols(j)
                    ncols = q1 - q0
                    for hp in range(2):
                        ps = score_ps.pop((j, hp))
                        ps_v = ps.rearrange("p (a x) -> p a x", a=2)
                        nc.scalar.activation(
                            eb[:, jj, hp, :, col0:col0 + ncols],
                            ps_v[:, :, col0:col0 + ncols],
                            mybir.ActivationFunctionType.Exp)

                def masks_chunk(j):
                    b, jj = j // 4, j % 4
                    eb = exp_blocks[b]
                    eng = nc.gpsimd if j >= 12 else nc.vector
                    # band masks for this chunk's side slots (keep-mask mul)
                    if j <= NT - 2:
                        sl = eb[:, jj, :, :, 256:384]
                        mb = (c2[:, C2_ML:C2_ML + 128].unsqueeze(1).unsqueeze(1)
                              .broadcast_to((128, 2, 2, 128)))
                        eng.tensor_mul(sl, sl, mb)
                    if j >= 1:
                        sl = eb[:, jj, :, :, 0:128]
                        mb = (c2[:, C2_MR:C2_MR + 128].unsqueeze(1).unsqueeze(1)
                              .broadcast_to((128, 2, 2, 128)))
                        eng.tensor_mul(sl, sl, mb)

                def attnv(t, hp):
                    cs = [c for c in (t - 1, t, t + 1) if 0 <= c < NT]
                    shape = [128, 130] if hp == 0 else [128, 386]
                    pa = pmix.tile(shape, F32, tag="mix", name=f"pa{t}_{hp}")
                    pa_tiles[(t, hp)] = pa
                    for a in range(2):
                        for i, cj in enumerate(cs):
                            slot = t - cj + 1
                            eb = exp_blocks[cj // 4]
                            nc.tensor.matmul(
                                pa[:, a * 65:(a + 1) * 65],
                                lhsT=eb[:, cj % 4, hp, a,
                                        slot * 128:(slot + 1) * 128],
                                rhs=v4[:, hp * 2 + a, cj, :],
                                start=(i == 0), stop=(i == len(cs) - 1),
                            )

                def normalize(t, hp):
                    pa = pa_tiles[(t, hp)]
                    rcp = rcpp.tile([128, 2], F32, tag="rcp")
                    nc.vector.reciprocal_approx_fast(rcp[:], pa[:, 64:130:65])
                    aq = aqp.tile([128, 2, 64], F32, tag="aq")
                    for a in range(2):
                        nc.vector.tensor_scalar_mul(
                            aq[:, a, :], pa[:, a * 65:a * 65 + 64],
                            rcp[:, a:a + 1])
                    return aq

                def transp(t, hp, aq):
                    pa = pa_tiles[(t, 1)]
                    nc.tensor.transpose(pa[:, 130 + hp * 128:258 + hp * 128],
                                        aq[:], ident[:])

                def attcopy(t):
                    att = attp.tile([128, 2, 128], BF, tag="att",
                                    name=f"att{t}")
                    att_tiles[t] = att
                    eng = nc.scalar if t >= 12 else nc.vector
                    if eng is nc.scalar:
                        nc.scalar.copy(att[:].rearrange("p h q -> p (h q)"),
                                       pa_tiles[(t, 1)][:, 130:386])
                    else:
                        nc.vector.tensor_copy(
                            att[:].rearrange("p h q -> p (h q)"),
                            pa_tiles[(t, 1)][:, 130:386])

                def outproj(t):
                    ps_o = pprod.tile([128, 512], F32, tag="prod", name=f"o{t}")
                    att = att_tiles.pop(t)
                    for hp in range(2):
                        nc.tensor.matmul(
                            ps_o[:],
                            lhsT=att[:, hp, :],
                            rhs=c2[:, C2_OW + hp * 512:C2_OW + (hp + 1) * 512],
                            start=(hp == 0), stop=(hp == 1),
                        )
                    del pa_tiles[(t, 0)], pa_tiles[(t, 1)]
                    return ps_o

                def outcopy_dma(t, ps_o):
                    if t >= 12:
                        nc.scalar.copy(osb[:, t % 4, :], ps_o[:])
                    else:
                        nc.vector.tensor_copy(osb[:, t % 4, :], ps_o[:])
                    if t % 2 == 1:
                        half = t // 2
                        nc.sync.dma_start(
                            out=out_d[half * 256:(half + 1) * 256, :].rearrange(
                                "(s p) f -> p s f", s=2),
                            in_=osb[:, (t % 4) - 1:(t % 4) + 1, :],
                        )

                # ------- software-pipelined step schedule -------
                # lags: qk/v production at s, rope rot at s+1, rope add at
                # s+2; scores chunk j at j+5, exp+masks at j+6; attention
                # tile t at t+8 (finish of t-1 interleaved).  Emission order
                # within a step puts each engine's independent work first.
                aqs = {}

                def attn_step(t):
                    tp = t - 1
                    if tp >= 0:
                        transp(tp, 0, aqs.pop((tp, 0)))
                        transp(tp, 1, aqs.pop((tp, 1)))
                        attcopy(tp)
                    attnv(t, 0)
                    aqs[(t, 0)] = normalize(t, 0)
                    if tp >= 0:
                        ps_o = outproj(tp)
                        outcopy_dma(tp, ps_o)
                    attnv(t, 1)
                    aqs[(t, 1)] = normalize(t, 1)

                def attn_last(t):
                    transp(t, 0, aqs.pop((t, 0)))
                    transp(t, 1, aqs.pop((t, 1)))
                    attcopy(t)
                    ps_o = outproj(t)
                    outcopy_dma(t, ps_o)

                # compressed-head schedule: production m-groups 0..7 run
                # two per step (steps 0..3), 8..15 one per step (4..11);
                # rope rot/add trail the group by 1/2 steps; scores chunk j
                # at j+4, exp+masks at j+5, attention tile t at t+6.
                step_of = [g // 2 if g < 8 else g - 4 for g in range(16)]
                rots = {}
                adds = {}
                for g in range(16):
                    rots.setdefault(step_of[g] + 1, []).append(g)
                    adds.setdefault(step_of[g] + 2, []).append(g)
                for s in range(23):
                    for g in adds.get(s, []):
                        rope_add(g)
                    for g in rots.get(s, []):
                        rope_rot(g)
                    je = s - 5
                    if 0 <= je < NT:
                        exp_chunk(je)
                        masks_chunk(je)
                    gs = [g for g in range(16) if step_of[g] == s]
                    for g in gs:
                        qk_mms(g)
                    for g in gs:
                        v_production(g)
                    j = s - 4
                    if 0 <= j < NT:
                        scores_chunk(j)
                    t = s - 6
                    if 0 <= t < NT:
                        attn_step(t)
                    if s == 22:
                        attn_last(15)

                if KDEBUG:
                    for b in range(4):
                        nc.sync.dma_start(out=dbg_exp[b], in_=exp_blocks[b][:])
                    nc.sync.dma_start(out=dbg_qT[:], in_=qT[:])
                    nc.sync.dma_start(out=dbg_kT[:], in_=kT[:])
                    nc.sync.dma_start(out=dbg_v4[:], in_=v4[:])

            if rep_ctx is not None:
                rep_ctx.__exit__(None, None, None)

    nc.compile()
    return nc


# ---------------- host prep + run + gather ----------------
def _get_state():
    if "nc" not in _CACHE:
        _CACHE["nc"] = build_nc()
    return _CACHE


def make_in_maps(x, Wqkv_w, out_w):
    halves = [_prep_consts(Wqkv_w, out_w, h) for h in range(2)]
    ident32 = np.eye(128, dtype=np.float32)
    in_maps = []
    for core in range(N_CORES):
        n, half = core // 2, core % 2
        c1, c2 = halves[half]
        xT = _bf16(x[n].T).reshape(4, 128, T).transpose(1, 0, 2)  # [128, 4, T]
        in_maps.append({
            "c1": c1, "c2": c2,
            "x0": np.ascontiguousarray(xT[:, :, 0:512]),
            "xr": np.ascontiguousarray(xT[:, :, 512:]),
            "ident32": ident32,
        })
    return in_maps


def gather(results, out_b, dtype):
    outs = []
    for n in range(N_BATCH):
        o = (results[2 * n]["out"].astype(np.float32)
             + results[2 * n + 1]["out"].astype(np.float32)
             + out_b[None, :])
        outs.append(o)
    return np.stack(outs).astype(dtype, copy=False)


def kernel(x, Wqkv_w, out_w, out_b):
    x = np.asarray(x)
    st = _get_state()
    in_maps = make_in_maps(x, np.asarray(Wqkv_w), np.asarray(out_w))
    res = bass_utils.run_bass_kernel_spmd(
        st["nc"], in_maps, core_ids=list(range(N_CORES)))
    return gather(res.results, np.asarray(out_b), x.dtype)


# revision 40
# speedup vs baseline: 1.0499x; 1.0499x over previous
"""Banded (sliding-window) multi-head attention for Trainium2, 8 NeuronCores.

Problem: x[4, 2048, 512] -> QKV proj -> RoPE -> banded attention
(window [q-127, q+128]) -> out proj.  See reference.py.

Sharding: (batch n, head-half) -> 8 cores.  Each core computes 4 heads of one
batch end-to-end and a partial out-projection (contraction over its 256 hidden
dims); host gather sums the two partials per batch and adds the bias.

Pipeline (all matmuls bf16 with fp32 PSUM accumulation):
  - 6 batched input DMAs (constants host-packed into two [128, ~4.3K] tensors).
  - qkT = Wqk^T.T @ xT in psum; raw copy to SBUF (ACT/DVE); RoPE via the
    identity rot(u)*sin == rot(u*sin): t1 = raw*cos (Pool), psr = R @ raw
    (PE), t2 = psr*sin (DVE), qkT/kT = t1 + t2 (Pool).
  - v token-major via xT.T @ WvT; ones column gives softmax denominators.
  - scores chunk-grouped: per (key chunk j, head pack, head a) one matmul
    kT_j^T.T @ qT[3-tile window], N<=384, psum [128, 2a, 512pad] 2-bank.
  - exp per (chunk, hp) over both heads in one strided ACT call; band masks
    as broadcast keep-mask multiplies (DVE; late chunks on Pool).
  - attn[q, 65] accumulated over 3 chunks; transpose written in-place into
    the dead tail of the attn psum bank; out proj -> bf16 out, paired DMAs.
  - Software-pipelined emission: compressed production head (2 m-groups per
    step, head v-psum from the attention pool while it idles), per-engine
    queue shaping, attention trailing production by 6 steps.
"""

import numpy as np
import ml_dtypes

import concourse.bass as bass
import concourse.bacc as bacc
import concourse.mybir as mybir
import concourse.tile as tile
from concourse import bass_utils

# ---------------- problem constants (hardcoded per contract) ----------------
N_BATCH = 4
T = 2048
D_MODEL = 512
NHEAD = 8
HEAD_DIM = 64           # also rotary dim
WIN_LO, WIN_HI = 127, 128
N_CORES = 8

NT = T // 128           # 16 query tiles / key chunks of 128
BF = mybir.dt.bfloat16
F32 = mybir.dt.float32

# c1 column layout: wqk [4x512] | wv [4x256] | cos0 [512] | sin0 [512] | rt [128]
C1_WQK, C1_WV, C1_COS, C1_SIN, C1_RT = 0, 2048, 3072, 3584, 4096
C1_COLS = 4224
# c2 layout: ow [2x512] | cos rest [1536] | sin rest [1536] | maskL | maskR
C2_OW, C2_COS, C2_SIN, C2_ML, C2_MR = 0, 1024, 2560, 4096, 4224
C2_COLS = 4352

_CACHE = {}


# ---------------- host-side constant prep ----------------
def _bf16(a):
    return np.ascontiguousarray(a, dtype=np.float32).astype(ml_dtypes.bfloat16)


def _rope_tables():
    # row p of a 128-partition head-pack corresponds to head dim d = p % 64
    d_idx = np.arange(128) % HEAD_DIM
    f_idx = d_idx % (HEAD_DIM // 2)
    invf = 1.0 / (10000.0 ** (np.arange(0, HEAD_DIM, 2, dtype=np.float32) / HEAD_DIM))
    ang = np.arange(T, dtype=np.float32)[None, :] * invf[f_idx][:, None]  # [128, T]
    return np.cos(ang, dtype=np.float32), np.sin(ang, dtype=np.float32)


def _rot_matrix_T():
    # ps_rot = R @ t2 with R the rotate_half signed permutation (per 64-dim
    # head); lhsT convention means we pass R.T
    R = np.zeros((128, 128), np.float32)
    for p in range(128):
        if p % 64 < 32:
            R[p, p + 32] = -1.0
        else:
            R[p, p - 32] = 1.0
    return R.T


def _prep_consts(Wqkv_w, out_w, half):
    hs = [half * 4 + i for i in range(4)]
    Wq = Wqkv_w[0 * D_MODEL:1 * D_MODEL].reshape(NHEAD, HEAD_DIM, D_MODEL)[hs]
    Wk = Wqkv_w[1 * D_MODEL:2 * D_MODEL].reshape(NHEAD, HEAD_DIM, D_MODEL)[hs]
    Wv = Wqkv_w[2 * D_MODEL:3 * D_MODEL].reshape(NHEAD, HEAD_DIM, D_MODEL)[hs]
    Wq = Wq * np.float32(1.0 / np.sqrt(HEAD_DIM))     # fold attention scale
    # feat order: q(h0,h1), q(h2,h3), k(h0,h1), k(h2,h3)
    wqk = np.concatenate([Wq.reshape(2, 128, D_MODEL), Wk.reshape(2, 128, D_MODEL)], 0)
    wqkT = wqk.reshape(512, D_MODEL).T.reshape(4, 128, 512)       # [xfeat c, 128, feat]
    wvT = Wv.reshape(256, D_MODEL).T.reshape(4, 128, 256)
    owT = out_w[:, half * 256:(half + 1) * 256].T.reshape(2, 128, 512)

    cos, sin = _rope_tables()
    rtT = _rot_matrix_T()
    c1 = np.concatenate([
        wqkT.transpose(1, 0, 2).reshape(128, 2048),
        wvT.transpose(1, 0, 2).reshape(128, 1024),
        cos[:, 0:512], sin[:, 0:512], rtT,
    ], axis=1)
    kp = np.arange(128)[:, None]
    qp = np.arange(128)[None, :]
    c2 = np.concatenate([
        owT.transpose(1, 0, 2).reshape(128, 1024),
        cos[:, 512:], sin[:, 512:],
        (kp >= qp + 1).astype(np.float32),   # left-role keep mask
        (kp <= qp).astype(np.float32),       # right-role keep mask
    ], axis=1)
    assert c1.shape[1] == C1_COLS and c2.shape[1] == C2_COLS
    return _bf16(c1), _bf16(c2)


# ---------------- bass program ----------------
def build_nc(reps=1):
    """reps>1 repeats the whole kernel body (timing harness only)."""
    import os
    KDEBUG = os.environ.get("KDEBUG") == "1"
    PHASE = os.environ.get("KBUILD_PHASE", "full")  # prod | noattn | full
    nc = bacc.Bacc("TRN2", debug=False, enable_asserts=False,
                   target_bir_lowering=False)

    c1_d = nc.dram_tensor("c1", [128, C1_COLS], BF, kind="ExternalInput")
    c2_d = nc.dram_tensor("c2", [128, C2_COLS], BF, kind="ExternalInput")
    x0_d = nc.dram_tensor("x0", [128, 4, 512], BF, kind="ExternalInput")
    xr_d = nc.dram_tensor("xr", [128, 4, 1536], BF, kind="ExternalInput")
    id_d = nc.dram_tensor("ident32", [128, 128], F32, kind="ExternalInput")
    out_d = nc.dram_tensor("out", [T, 512], BF, kind="ExternalOutput")
    if KDEBUG:
        dbg_qT = nc.dram_tensor("dbg_qT", [128, 2, T], BF, kind="ExternalOutput")
        dbg_kT = nc.dram_tensor("dbg_kT", [128, 2, T], BF, kind="ExternalOutput")
        dbg_v4 = nc.dram_tensor("dbg_v4", [128, 4, NT, 65], BF, kind="ExternalOutput")
        dbg_exp = nc.dram_tensor("dbg_exp", [4, 128, 4, 2, 2, 384], BF, kind="ExternalOutput")

    with tile.TileContext(nc) as tc:
        with (
            tc.tile_pool(name="persist", bufs=1) as pers,
            tc.tile_pool(name="rawp", bufs=3) as rawp,
            tc.tile_pool(name="t1p", bufs=3) as t1p,
            tc.tile_pool(name="t2p", bufs=3) as t2p,
            tc.tile_pool(name="rcpp", bufs=4) as rcpp,
            tc.tile_pool(name="aqp", bufs=6) as aqp,
            tc.tile_pool(name="attp", bufs=3) as attp,
            tc.tile_pool(name="expp", bufs=4) as expp,
            tc.tile_pool(name="pscore", bufs=2, space="PSUM") as pscore,
            tc.tile_pool(name="pprod", bufs=2, space="PSUM") as pprod,
            tc.tile_pool(name="pmix", bufs=2, space="PSUM") as pmix,
        ):
            # ------- persistent SBUF tensors -------
            c1 = pers.tile([128, C1_COLS], BF)
            c2 = pers.tile([128, C2_COLS], BF)
            ident = pers.tile([128, 128], F32)
            xT = pers.tile([128, 4, T], BF)
            qT = pers.tile([128, 2, T], BF)
            kT = pers.tile([128, 2, T], BF)
            v4 = pers.tile([128, 4, NT, 65], BF)
            osb = pers.tile([128, 4, 512], BF)

            def cos_ap(n):
                return (c1[:, C1_COS:C1_COS + 512] if n == 0
                        else c2[:, C2_COS + (n - 1) * 512:C2_COS + n * 512])

            def sin_ap(n):
                return (c1[:, C1_SIN:C1_SIN + 512] if n == 0
                        else c2[:, C2_SIN + (n - 1) * 512:C2_SIN + n * 512])

            rep_ctx = tc.For_i(0, reps, 1) if reps > 1 else None
            if rep_ctx is not None:
                rep_ctx.__enter__()
            for rep in range(1):
                # ------- input DMAs (few, large) -------
                # issue input DMAs from separate queues so descriptor
                # generation and transfers overlap from t=0
                nc.sync.dma_start(out=c1[:, 0:3072], in_=c1_d[:, 0:3072])
                nc.sync.dma_start(out=xT[:, :, 0:512], in_=x0_d[:])
                nc.sync.dma_start(out=xT[:, :, 512:T], in_=xr_d[:])
                nc.sync.dma_start(out=c1[:, 3072:], in_=c1_d[:, 3072:])
                nc.sync.dma_start(out=c2[:], in_=c2_d[:])
                nc.sync.dma_start(out=ident[:], in_=id_d[:])

                nc.vector.memset(v4[:, :, :, 64:65], 1.0)  # ones col -> sums

                # per-step state carried between pipeline stages
                raws = [None] * 16          # raw qk [128,512] sbuf bf16 per m-idx
                t1s = [None] * 16           # raw*cos
                exp_blocks = [None] * 4
                pa_tiles = {}               # (t, hp) -> psum attn tile
                att_tiles = {}              # t -> sbuf attnT tile

                # stage A: qk production matmuls + raw copy + cos mul
                def qk_mms(s):
                    n, m = s // 4, s % 4
                    nsl = slice(n * 512, (n + 1) * 512)
                    ps = pprod.tile([128, 512], F32, tag="prod", name=f"qk{s}")
                    for c in range(4):
                        nc.tensor.matmul(
                            ps[:],
                            lhsT=c1[:, C1_WQK + c * 512 + m * 128:
                                    C1_WQK + c * 512 + (m + 1) * 128],
                            rhs=xT[:, c, nsl],
                            start=(c == 0), stop=(c == 3),
                        )
                    raw = rawp.tile([128, 512], BF, tag="raw")
                    if s < 8:
                        nc.vector.tensor_copy(raw[:], ps[:])
                    else:
                        nc.scalar.copy(raw[:], ps[:])
                    t1 = t1p.tile([128, 512], BF, tag="t1")
                    nc.gpsimd.tensor_mul(t1[:], raw[:], cos_ap(n))
                    raws[s], t1s[s] = raw, t1

                # stage B (one step later): rotate on PE, sin mul on DVE
                t2s = [None] * 16

                def rope_rot(s):
                    n = s // 4
                    psr = pprod.tile([128, 512], F32, tag="prod", name=f"rot{s}")
                    nc.tensor.matmul(psr[:], lhsT=c1[:, C1_RT:C1_RT + 128],
                                     rhs=raws[s][:], start=True, stop=True)
                    t2 = t2p.tile([128, 512], BF, tag="t2")
                    nc.vector.tensor_mul(t2[:], psr[:], sin_ap(n))
                    t2s[s] = t2
                    raws[s] = None

                # stage C (two steps later): combine on Pool
                def rope_add(s):
                    n, m = s // 4, s % 4
                    nsl = slice(n * 512, (n + 1) * 512)
                    dest = qT[:, m, nsl] if m < 2 else kT[:, m - 2, nsl]
                    nc.gpsimd.tensor_add(dest, t1s[s][:], t2s[s][:])
                    t1s[s] = t2s[s] = None

                def v_production(t):
                    pool = pmix if t < 8 else pprod
                    ps = pool.tile([128, 512], F32,
                                   tag="mix" if t < 8 else "prod",
                                   name=f"v{t}")
                    for c in range(4):
                        nc.tensor.matmul(
                            ps[:, 0:256],
                            lhsT=xT[:, c, t * 128:(t + 1) * 128],
                            rhs=c1[:, C1_WV + c * 256:C1_WV + (c + 1) * 256],
                            start=(c == 0), stop=(c == 3),
                        )
                    nc.scalar.copy(
                        v4[:, :, t, 0:64],
                        ps[:, 0:256].rearrange("p (h d) -> p h d", h=4),
                    )

                # scores matmuls for key chunk j (exp runs next step)
                score_ps = {}

                def chunk_cols(j):
                    q0 = max(j - 1, 0) * 128
                    q1 = min(j + 2, NT) * 128
                    return q0, q1, q0 - (j - 1) * 128

                def scores_chunk(j):
                    q0, q1, col0 = chunk_cols(j)
                    ncols = q1 - q0
                    for hp in range(2):
                        ps = pscore.tile([128, 1024], F32, tag="score",
                                         name=f"sc{j}_{hp}")
                        score_ps[(j, hp)] = ps
                        for a in range(2):
                            rsl = slice(a * 64, (a + 1) * 64)
                            nc.tensor.matmul(
                                ps[:, a * 512 + col0:a * 512 + col0 + ncols],
                                lhsT=kT[rsl, hp, j * 128:(j + 1) * 128],
                                rhs=qT[rsl, hp, q0:q1],
                                start=True, stop=True,
                            )

                def exp_chunk(j):
                    b, jj = j // 4, j % 4
                    if jj == 0:
                        exp_blocks[b] = expp.tile([128, 4, 2, 2, 384], BF,
                                                  tag="exp", name=f"exp{b}")
                    eb = exp_blocks[b]
                    if KDEBUG and j == 0:
                        nc.vector.memset(eb[:, 0, :, :, 0:128], 0.0)
                    if KDEBUG and j == NT - 1:
                        nc.vector.memset(eb[:, 3, :, :, 256:384], 0.0)
                    q0, q1, col0 = chunk_cols(j)
                    ncols = q1 - q0
                    for hp in range(2):
                        ps = score_ps.pop((j, hp))
                        ps_v = ps.rearrange("p (a x) -> p a x", a=2)
                        nc.scalar.activation(
                            eb[:, jj, hp, :, col0:col0 + ncols],
                            ps_v[:, :, col0:col0 + ncols],
                            mybir.ActivationFunctionType.Exp)

                def masks_chunk(j):
                    b, jj = j // 4, j % 4
                    eb = exp_blocks[b]
                    eng = nc.gpsimd if j >= 12 else nc.vector
                    # band masks for this chunk's side slots (keep-mask mul)
                    if j <= NT - 2:
                        sl = eb[:, jj, :, :, 256:384]
                        mb = (c2[:, C2_ML:C2_ML + 128].unsqueeze(1).unsqueeze(1)
                              .broadcast_to((128, 2, 2, 128)))
                        eng.tensor_mul(sl, sl, mb)
                    if j >= 1:
                        sl = eb[:, jj, :, :, 0:128]
                        mb = (c2[:, C2_MR:C2_MR + 128].unsqueeze(1).unsqueeze(1)
                              .broadcast_to((128, 2, 2, 128)))
                        eng.tensor_mul(sl, sl, mb)

                def attnv(t, hp):
                    cs = [c for c in (t - 1, t, t + 1) if 0 <= c < NT]
                    shape = [128, 130] if hp == 0 else [128, 386]
                    pa = pmix.tile(shape, F32, tag="mix", name=f"pa{t}_{hp}")
                    pa_tiles[(t, hp)] = pa
                    for a in range(2):
                        for i, cj in enumerate(cs):
                            slot = t - cj + 1
                            eb = exp_blocks[cj // 4]
                            nc.tensor.matmul(
                                pa[:, a * 65:(a + 1) * 65],
                                lhsT=eb[:, cj % 4, hp, a,
                                        slot * 128:(slot + 1) * 128],
                                rhs=v4[:, hp * 2 + a, cj, :],
                                start=(i == 0), stop=(i == len(cs) - 1),
                            )

                def normalize(t, hp):
                    pa = pa_tiles[(t, hp)]
                    rcp = rcpp.tile([128, 2], F32, tag="rcp")
                    nc.vector.reciprocal_approx_fast(rcp[:], pa[:, 64:130:65])
                    aq = aqp.tile([128, 2, 64], F32, tag="aq")
                    for a in range(2):
                        nc.vector.tensor_scalar_mul(
                            aq[:, a, :], pa[:, a * 65:a * 65 + 64],
                            rcp[:, a:a + 1])
                    return aq

                def transp(t, hp, aq):
                    pa = pa_tiles[(t, 1)]
                    nc.tensor.transpose(pa[:, 130 + hp * 128:258 + hp * 128],
                                        aq[:], ident[:])

                def attcopy(t):
                    att = attp.tile([128, 2, 128], BF, tag="att",
                                    name=f"att{t}")
                    att_tiles[t] = att
                    eng = nc.scalar if t >= 12 else nc.vector
                    if eng is nc.scalar:
                        nc.scalar.copy(att[:].rearrange("p h q -> p (h q)"),
                                       pa_tiles[(t, 1)][:, 130:386])
                    else:
                        nc.vector.tensor_copy(
                            att[:].rearrange("p h q -> p (h q)"),
                            pa_tiles[(t, 1)][:, 130:386])

                def outproj(t):
                    ps_o = pprod.tile([128, 512], F32, tag="prod", name=f"o{t}")
                    att = att_tiles.pop(t)
                    for hp in range(2):
                        nc.tensor.matmul(
                            ps_o[:],
                            lhsT=att[:, hp, :],
                            rhs=c2[:, C2_OW + hp * 512:C2_OW + (hp + 1) * 512],
                            start=(hp == 0), stop=(hp == 1),
                        )
                    del pa_tiles[(t, 0)], pa_tiles[(t, 1)]
                    return ps_o

                def outcopy_dma(t, ps_o):
                    if t >= 12:
                        nc.scalar.copy(osb[:, t % 4, :], ps_o[:])
                    else:
                        nc.vector.tensor_copy(osb[:, t % 4, :], ps_o[:])
                    if t >= 14:
                        nc.sync.dma_start(
                            out=out_d[t * 128:(t + 1) * 128, :],
                            in_=osb[:, t % 4, :],
                        )
                    elif t % 2 == 1:
                        half = t // 2
                        nc.sync.dma_start(
                            out=out_d[half * 256:(half + 1) * 256, :].rearrange(
                                "(s p) f -> p s f", s=2),
                            in_=osb[:, (t % 4) - 1:(t % 4) + 1, :],
                        )

                # ------- software-pipelined step schedule -------
                # lags: qk/v production at s, rope rot at s+1, rope add at
                # s+2; scores chunk j at j+5, exp+masks at j+6; attention
                # tile t at t+8 (finish of t-1 interleaved).  Emission order
                # within a step puts each engine's independent work first.
                aqs = {}

                def attn_step(t):
                    tp = t - 1
                    attnv(t, 0)
                    if tp >= 0:
                        transp(tp, 0, aqs.pop((tp, 0)))
                        transp(tp, 1, aqs.pop((tp, 1)))
                        attcopy(tp)
                    aqs[(t, 0)] = normalize(t, 0)
                    if tp >= 0:
                        ps_o = outproj(tp)
                        outcopy_dma(tp, ps_o)
                    attnv(t, 1)
                    aqs[(t, 1)] = normalize(t, 1)

                def attn_last(t):
                    transp(t, 0, aqs.pop((t, 0)))
                    transp(t, 1, aqs.pop((t, 1)))
                    attcopy(t)
                    ps_o = outproj(t)
                    outcopy_dma(t, ps_o)

                # compressed-head schedule: production m-groups 0..7 run
                # two per step (steps 0..3), 8..15 one per step (4..11);
                # rope rot/add trail the group by 1/2 steps; scores chunk j
                # at j+4, exp+masks at j+5, attention tile t at t+6.
                step_of = [g // 2 if g < 8 else g - 4 for g in range(16)]
                rots = {}
                adds = {}
                for g in range(16):
                    rots.setdefault(step_of[g] + 1, []).append(g)
                    adds.setdefault(step_of[g] + 2, []).append(g)
                for s in range(23):
                    for g in adds.get(s, []):
                        rope_add(g)
                    for g in rots.get(s, []):
                        rope_rot(g)
                    je = s - 5
                    if 0 <= je < NT:
                        exp_chunk(je)
                        masks_chunk(je)
                    gs = [g for g in range(16) if step_of[g] == s]
                    for g in gs:
                        qk_mms(g)
                    for g in gs:
                        v_production(g)
                    j = s - 4
                    if 0 <= j < NT:
                        scores_chunk(j)
                    t = s - 6
                    if 0 <= t < NT:
                        attn_step(t)
                    if s == 22:
                        attn_last(15)

                if KDEBUG:
                    for b in range(4):
                        nc.sync.dma_start(out=dbg_exp[b], in_=exp_blocks[b][:])
                    nc.sync.dma_start(out=dbg_qT[:], in_=qT[:])
                    nc.sync.dma_start(out=dbg_kT[:], in_=kT[:])
                    nc.sync.dma_start(out=dbg_v4[:], in_=v4[:])

            if rep_ctx is not None:
                rep_ctx.__exit__(None, None, None)

    nc.compile()
    return nc


# ---------------- host prep + run + gather ----------------
def _get_state():
    if "nc" not in _CACHE:
        _CACHE["nc"] = build_nc()
    return _CACHE


def make_in_maps(x, Wqkv_w, out_w):
    halves = [_prep_consts(Wqkv_w, out_w, h) for h in range(2)]
    ident32 = np.eye(128, dtype=np.float32)
    in_maps = []
    for core in range(N_CORES):
        n, half = core // 2, core % 2
        c1, c2 = halves[half]
        xT = _bf16(x[n].T).reshape(4, 128, T).transpose(1, 0, 2)  # [128, 4, T]
        in_maps.append({
            "c1": c1, "c2": c2,
            "x0": np.ascontiguousarray(xT[:, :, 0:512]),
            "xr": np.ascontiguousarray(xT[:, :, 512:]),
            "ident32": ident32,
        })
    return in_maps


def gather(results, out_b, dtype):
    outs = []
    for n in range(N_BATCH):
        o = (results[2 * n]["out"].astype(np.float32)
             + results[2 * n + 1]["out"].astype(np.float32)
             + out_b[None, :])
        outs.append(o)
    return np.stack(outs).astype(dtype, copy=False)


def kernel(x, Wqkv_w, out_w, out_b):
    x = np.asarray(x)
    st = _get_state()
    in_maps = make_in_maps(x, np.asarray(Wqkv_w), np.asarray(out_w))
    res = bass_utils.run_bass_kernel_spmd(
        st["nc"], in_maps, core_ids=list(range(N_CORES)))
    return gather(res.results, np.asarray(out_b), x.dtype)
